# revision 12
# baseline (speedup 1.0000x reference)
"""Ragged -> padded batch scatter (BatchedSequences) on 8 TRN2 NeuronCores.

Reference semantics: rows of concatenated_sequences [T, F] are scattered into
a zero-padded output [B, max_sl, F] according to per-sequence lengths.

Strategy (pure data movement, memory-bound):
  - All sequence lengths are multiples of 64, so work in 64-row "chunks"
    (64*512 f32 = 128 KiB each, contiguous both in input and output).
  - Shard sequences across 8 cores with a balanced pairing so every core
    moves the same number of chunks -> a single uniform SPMD program.
  - Per core: stream contiguous chunk-groups HBM->SBUF with direct DMA,
    then indirect-scatter each chunk to its destination chunk in the padded
    per-core output, driven by a host-computed int32 index tensor.
  - Padding stays zero because run_bass_kernel_spmd pre-zeroes / donates
    zero-filled ExternalOutput buffers.
"""

from contextlib import ExitStack

import numpy as np

import concourse.bass as bass
import concourse.mybir as mybir
from concourse.bass_utils import run_bass_kernel_spmd

B = 32
F = 512
MAX_SL = 4096
NCORES = 8
SEQ_PER_CORE = B // NCORES
CHUNK = 64                       # rows per length-granularity chunk
SUP_EL = 8192                    # f32 elements per superchunk = 32 KiB
SUP_ROWS = SUP_EL // F           # 16 rows per superchunk
OUT_CHUNKS = SEQ_PER_CORE * MAX_SL // CHUNK   # 256 data chunks per core
OUT_SUPS = (OUT_CHUNKS + 1) * CHUNK // SUP_ROWS  # incl. trash chunk -> 1028
TRASH_SUP = OUT_CHUNKS * CHUNK // SUP_ROWS       # 1024
SUPS_GROUP = 128                 # superchunks per instruction (one/partition)

_NC_CACHE: dict[int, bass.Bass] = {}


NSLOTS = 5  # staging buffers


def _group_plan(n_rows: int):
    """Split n_rows into groups of (rows, extent_rows). Each group occupies
    rows/extent <= 128 partitions. Workhorse groups are 2048 rows at
    16-row extents (32 KB/partition); the final groups use smaller extents so
    the end-of-kernel load->scatter chain drains quickly. n_rows must be a
    multiple of SUP_ROWS."""
    plan = []
    rem = n_rows
    while rem >= 2048:
        plan.append((2048, 16))
        rem -= 2048
    if rem:
        e = 8 if rem // 8 <= 128 else 16
        plan.append((rem, e))
    return plan


def _build_nc(n_sups: int) -> bass.Bass:
    """Uniform per-core program: scatter superchunks of x into superchunks of
    y selected by dst. y has one extra trash chunk for padded (unused) source
    superchunks.

    HW indirect-DMA contract (probed): offsets live one-per-partition
    ([P, 1] int32); for index p the DMA moves in_'s partition-p free extent
    (E elements) to out.flat[idx[p]*coef : +E], where coef is the product of
    the out-AP dims after the indirect axis. We keep E == coef per group.

    Raw Bass (no Tile): DMA queue instructions only support a single attached
    sync-wait, so all waits are standalone sequencer instructions, and each
    DMA signals its own dedicated semaphore. Scatters are not ordered among
    themselves (destination superchunks are disjoint by construction)."""
    nc = bass.Bass()
    n_rows = n_sups * SUP_ROWS
    x = nc.declare_dram_parameter("x", [n_rows, F], mybir.dt.float32, isOutput=False)
    plan = _group_plan(n_rows)
    ng = len(plan)

    dst = nc.declare_dram_parameter("dst", [128, ng], mybir.dt.int32, isOutput=False)
    y = nc.declare_dram_parameter(
        "y", [(OUT_CHUNKS + 1) * CHUNK, F], mybir.dt.float32, isOutput=True
    )

    with ExitStack() as ctx:
        stage = ctx.enter_context(
            nc.sbuf_tensor([128, NSLOTS * SUP_EL], mybir.dt.float32)
        )
        dst_t = ctx.enter_context(nc.sbuf_tensor([128, ng], mybir.dt.int32))
        sem_dst = ctx.enter_context(nc.semaphore("sem_dst"))
        sem_load = [ctx.enter_context(nc.semaphore(f"sem_load{g}")) for g in range(ng)]
        sem_scat = [ctx.enter_context(nc.semaphore(f"sem_scat{g}")) for g in range(ng)]
        block = ctx.enter_context(nc.Block())

        @block.scalar
        def _(scalar):
            # tiny index-table load on the second HWDGE ring, overlapping the
            # first data load on sync
            scalar.dma_start(out=dst_t[:, :], in_=dst[:, :]).then_inc(sem_dst, 16)

        @block.sync
        def _(sync):
            r0 = 0
            for g, (rows, ext) in enumerate(plan):
                parts = rows // ext
                if g >= NSLOTS:
                    sync.wait_ge(sem_scat[g - NSLOTS], 16)
                slot = g % NSLOTS
                xin = x[r0 : r0 + rows, :].rearrange("(p q) f -> p (q f)", p=parts)
                sync.dma_start(
                    out=stage[:parts, slot * SUP_EL : slot * SUP_EL + ext * F],
                    in_=xin,
                ).then_inc(sem_load[g], 16)
                r0 += rows

        @block.gpsimd
        def _(gp):
            gp.wait_ge(sem_dst, 16)
            for g, (rows, ext) in enumerate(plan):
                parts = rows // ext
                slot = g % NSLOTS
                yv = y.rearrange("(n e) f -> n (e f)", e=ext)
                gp.wait_ge(sem_load[g], 16)
                gp.indirect_dma_start(
                    out=yv[:, :],
                    out_offset=bass.IndirectOffsetOnAxis(
                        ap=dst_t[:parts, g : g + 1], axis=0
                    ),
                    in_=stage[:parts, slot * SUP_EL : slot * SUP_EL + ext * F],
                    in_offset=None,
                ).then_inc(sem_scat[g], 16)
            for g in range(ng):
                gp.wait_ge(sem_scat[g], 16)
    return nc


def _plan(L: np.ndarray):
    """Assign SEQ_PER_CORE sequences to each core, balanced.

    Returns (groups, n_chunks) where groups[k] is the list of sequence ids on
    core k and n_chunks is the max chunk count across cores (cores with fewer
    chunks pad their dst with the trash chunk)."""
    assert len(L) == B
    # Pairing (i, B-1-i) balances linearly-decaying lengths exactly; fall back
    # to a greedy LPT assignment for arbitrary lengths.
    pair_groups = [
        [k, B - 1 - k, k + NCORES, B - 1 - k - NCORES] for k in range(NCORES)
    ]
    totals = [sum(int(L[s]) for s in g) for g in pair_groups]
    if max(totals) - min(totals) <= 2 * CHUNK:
        groups = pair_groups
    else:
        order = np.argsort(-L)
        groups = [[] for _ in range(NCORES)]
        gtot = [0] * NCORES
        for s in order:
            k = min(
                (k for k in range(NCORES) if len(groups[k]) < SEQ_PER_CORE),
                key=lambda k: gtot[k],
            )
            groups[k].append(int(s))
            gtot[k] += int(L[s])
    n_chunks = max(sum(int(L[s]) for s in g) for g in groups) // CHUNK
    return groups, n_chunks


def _host_fallback(S, L, max_sl):
    out = np.zeros((len(L), max_sl, S.shape[1]), dtype=S.dtype)
    off = 0
    for b, ln in enumerate(L):
        out[b, :ln] = S[off : off + ln]
        off += ln
    return out


def _prepare(S, L):
    """Host planning: returns (nc, in_maps, groups)."""
    offsets = np.zeros(B + 1, dtype=np.int64)
    np.cumsum(L, out=offsets[1:])

    groups, n_chunks = _plan(L)
    n_sups = n_chunks * CHUNK // SUP_ROWS
    n_rows = n_sups * SUP_ROWS
    plan = _group_plan(n_rows)
    ng = len(plan)
    trash_row = OUT_CHUNKS * CHUNK  # first row of the trash chunk

    in_maps = []
    for k in range(NCORES):
        xs = []
        for j, s in enumerate(groups[k]):
            ln = int(L[s])
            xs.append(S[offsets[s] : offsets[s] + ln])
        rows = sum(x.shape[0] for x in xs)
        pad_rows = n_rows - rows
        if pad_rows:
            xs.append(np.zeros((pad_rows, F), dtype=np.float32))
        x_k = np.concatenate(xs, axis=0)

        # destination out-row for every source row (pads -> trash chunk)
        dest_row = np.full(n_rows, trash_row, dtype=np.int64)
        pos = 0
        for j, s in enumerate(groups[k]):
            ln = int(L[s])
            dest_row[pos : pos + ln] = j * MAX_SL + np.arange(ln)
            pos += ln

        # dst layout [128, ng]: column g holds group g's per-partition indices
        # in units of that group's extent
        dst_k = np.zeros((128, ng), dtype=np.int32)
        r0 = 0
        for g, (grows, ext) in enumerate(plan):
            parts = grows // ext
            src = r0 + np.arange(parts) * ext
            dst_k[:parts, g] = dest_row[src] // ext
            r0 += grows
        in_maps.append({"x": x_k, "dst": np.ascontiguousarray(dst_k)})

    if n_sups not in _NC_CACHE:
        _NC_CACHE[n_sups] = _build_nc(n_sups)
    return _NC_CACHE[n_sups], in_maps, groups


def _assemble(results, groups):
    out = np.empty((B, MAX_SL, F), dtype=np.float32)
    for k in range(NCORES):
        yk = np.asarray(results[k]["y"])[: SEQ_PER_CORE * MAX_SL].reshape(
            SEQ_PER_CORE, MAX_SL, F
        )
        for j, s in enumerate(groups[k]):
            out[s] = yk[j]
    return out


def kernel(concatenated_sequences, sequence_lengths, max_sl):
    S = np.ascontiguousarray(np.asarray(concatenated_sequences, dtype=np.float32))
    L = np.asarray(sequence_lengths).reshape(-1).astype(np.int64)
    max_sl = int(np.asarray(max_sl))

    if (
        max_sl != MAX_SL
        or len(L) != B
        or S.shape[1] != F
        or int(L.sum()) != S.shape[0]
        or np.any(L % CHUNK)
        or np.any(L < 0)
        or np.any(L > max_sl)
    ):
        return _host_fallback(S, L, max_sl)

    nc, in_maps, groups = _prepare(S, L)
    res = run_bass_kernel_spmd(nc, in_maps, list(range(NCORES))).results
    return _assemble(res, groups)
